# revision 4
# baseline (speedup 1.0000x reference)
"""Bass/Trainium2 kernel for nn_CSEM sparse_attention problem (v2).

Sharding: 8 cores = 4 samples x 2 spatial halves (bottom half vertically
flipped on host so all cores run an identical NEFF).

v2 vs baseline:
- conv0: (cin, kx{0,1}) folded into a 128-partition contraction tile; x1|x3
  merged into one 128-wide output tile (12 -> ~6 matmuls/block).
- conv1: (c, ky) folded into 5 contraction tiles T0..T4 (row-shifted copies
  of t), 128-aligned output tiles -> 15 full matmuls/block-tile vs 18.
- single AllReduce: S' = q'k'^T unnormalized + sumsq(q/k) as 2 extra f32
  columns; normalization applied to the 192x192 logits post-collective.
  The collective overlaps conv1's v output tiles.
- sumsq on Act engine (Square + accum_out), pools/bilinear chunked to
  overlap conv0, DMAs issued from the idle SP engine, border-only memsets.
"""

import numpy as np
import ml_dtypes

import concourse.bass as bass
import concourse.mybir as mybir
import concourse.tile as tile
from concourse.bass_utils import run_bass_kernel_spmd
from concourse.masks import make_identity

BF16 = mybir.dt.bfloat16
F32 = mybir.dt.float32
BN_EPS = 1e-5

CIN, C, C3 = 64, 192, 576
W, WP = 96, 98
XR = 54          # x slab rows (1 zero pad + 53 data)
YR = 52          # y rows computed locally (0..51)
TR = 50          # t rows 0..49
QR = 49          # conv1 v rows 0..48
SR = 48          # rows feeding S partial
OR_ = 48         # final output rows per core
NCH = SR * W // 128   # 36 transpose chunks
GROUPS = [[0, 1], [2, 3], [4, 5], [6, 7]]

# conv1 contraction tiles: partition P = ky*192 + c, tiles of 128
#   T0: ky0 c0..127 | T1: ky0 c128..191 + ky1 c0..63 | T2: ky1 c64..191
#   T3: ky2 c0..127 | T4: ky2 c128..191 (64 partitions)
T_SPECS = [128, 128, 128, 128, 64]


def _split_waits(nc, limit=1):
    """This walrus build rejects instructions carrying more than one sem-wait
    command. Spread extra waits onto same-engine ENGINE_NOPs inserted just
    before the offending instruction (semantically identical: the engine
    blocks on each wait in program order)."""
    ctr = [0]
    for f in nc.m.functions:
        for blk in f.blocks:
            il = blk.instructions
            new = []
            for inst in il:
                si = inst.sync_info
                waits = list(si.on_wait) if (si and si.on_wait) else []
                if len(waits) > limit:
                    for w in waits[:-limit]:
                        ctr[0] += 1
                        nop = mybir.InstNoOp(name=f"WNOP-{ctr[0]}")
                        nop.engine = inst.engine
                        nop.sync_info = mybir.SyncInfo(on_wait=[w], on_update=[])
                        new.append(nop)
                    si.on_wait = waits[-limit:]
                new.append(inst)
            il[:] = new


def _row_blocks(nrows, per=5):
    out, r = [], 0
    while r < nrows:
        n = min(per, nrows - r)
        out.append((r, n))
        r += n
    return out


def build_nc():
    nc = bass.Bass()

    xs_d = nc.declare_dram_parameter("xs", [64, XR * WP], BF16, isOutput=False)
    w0f_d = nc.declare_dram_parameter("w0f", [128, 3 * C], BF16, isOutput=False)
    w0g_d = nc.declare_dram_parameter("w0g", [128, 3 * C], BF16, isOutput=False)
    sb0_d = nc.declare_dram_parameter("sb0p", [C, 2], F32, isOutput=False)
    w1_d = [nc.declare_dram_parameter(f"w1t{i}", [T_SPECS[i], 3 * C3], BF16,
                                      isOutput=False) for i in range(5)]
    sb1_d = nc.declare_dram_parameter("sb1", [C3, 2], F32, isOutput=False)
    w2a_d = nc.declare_dram_parameter("w2da", [128, 9 * 128], BF16, isOutput=False)
    w2f_d = nc.declare_dram_parameter("w2fb", [128, 3 * 64], BF16, isOutput=False)
    w2g_d = nc.declare_dram_parameter("w2gb", [128, 3 * 64], BF16, isOutput=False)
    b2_d = nc.declare_dram_parameter("b2v", [C, 1], F32, isOutput=False)
    tmp_d = nc.declare_dram_parameter("tempv", [1, 1], F32, isOutput=False)
    yout = nc.declare_dram_parameter("yout", [C, OR_ * W], F32, isOutput=True)

    cc2i = nc.dram_tensor("cc2i", [C, 194], F32)
    cc2o = nc.dram_tensor("cc2o", [C, 194], F32)

    with tile.TileContext(nc) as tc:
        _body(nc, tc, xs_d, w0f_d, w0g_d, sb0_d, w1_d, sb1_d, w2a_d, w2f_d,
              w2g_d, b2_d, tmp_d, yout, cc2i, cc2o)
    _split_waits(nc)
    return nc


def _body(nc, tc, xs_d, w0f_d, w0g_d, sb0_d, w1_d, sb1_d, w2a_d, w2f_d,
          w2g_d, b2_d, tmp_d, yout, cc2i, cc2o):
    import contextlib
    ctx = contextlib.ExitStack()
    P = ctx.enter_context(tc.tile_pool(name="persist", bufs=1))
    ev = ctx.enter_context(tc.tile_pool(name="evac", bufs=3))

    # ---- persistent SBUF ----
    xfold = P.tile([128, XR * WP], BF16, tag="xfold")
    w0f = P.tile([128, 3 * C], BF16, tag="w0f")
    w0g = P.tile([128, 3 * C], BF16, tag="w0g")  # data in partitions 64..127
    w1s = [P.tile([T_SPECS[i], 3 * C3], BF16, tag=f"w1s{i}", name=f"w1s{i}")
           for i in range(5)]
    w2da = P.tile([128, 9 * 128], BF16, tag="w2da")
    w2fb = P.tile([128, 3 * 64], BF16, tag="w2fb")
    w2gb = P.tile([128, 3 * 64], BF16, tag="w2gb")  # data at partitions 64..
    sc0a = P.tile([128, 2], F32, tag="sc0a")
    sc0b = P.tile([64, 2], F32, tag="sc0b")
    scp = [128, 128, 128, 128, 64]
    sc1 = [P.tile([scp[i], 2], F32, tag=f"sc1_{i}", name=f"sc1_{i}")
           for i in range(5)]
    b2a = P.tile([128, 1], F32, tag="b2a")
    b2b = P.tile([64, 1], F32, tag="b2b")
    tmps = P.tile([128, 1], F32, tag="tmps")

    # x fold: p0..63 = x shifted right one col (kx=0 operand), p64..127 = x.
    # Row-chunked so conv0 starts on the first half.
    xfv0 = xfold.rearrange("p (r w) -> p r w", w=WP)
    for (a, b) in ((0, 12), (12, 32), (32, XR)):
        nc.sync.dma_start(out=xfold[64:128, a * WP:b * WP],
                          in_=xs_d[:, a * WP:b * WP])
        nc.sync.dma_start(
            out=xfv0[0:64, a:b, 1:98],
            in_=bass.AP(tensor=xs_d, offset=a * WP,
                        ap=[[XR * WP, 64], [WP, b - a], [1, 97]]))
        if a == 0:
            nc.sync.dma_start(out=w0f[:], in_=w0f_d[:])
            nc.sync.dma_start(out=w0g[:], in_=w0g_d[:])
    for i in range(5):
        nc.sync.dma_start(out=w1s[i][:], in_=w1_d[i][:])
    nc.sync.dma_start(out=w2da[:], in_=w2a_d[:])
    nc.sync.dma_start(out=w2fb[:], in_=w2f_d[:])
    nc.sync.dma_start(out=w2gb[:], in_=w2g_d[:])
    nc.sync.dma_start(out=sc0a[:], in_=sb0_d[0:128, :])
    nc.sync.dma_start(out=sc0b[:], in_=sb0_d[128:192, :])
    for i, (lo, hi) in enumerate([(0, 128), (128, 256), (256, 384),
                                  (384, 512), (512, 576)]):
        nc.sync.dma_start(out=sc1[i][:], in_=sb1_d[lo:hi, :])
    nc.sync.dma_start(out=b2a[:], in_=b2_d[0:128, :])
    nc.sync.dma_start(out=b2b[:], in_=b2_d[128:192, :])
    nc.sync.dma_start(
        out=tmps[:],
        in_=bass.AP(tensor=tmp_d, offset=0, ap=[[0, 128], [1, 1]]))

    ident = P.tile([128, 128], BF16, tag="ident")
    make_identity(nc, ident[:])
    identf = P.tile([128, 128], F32, tag="identf")
    make_identity(nc, identf[:])

    # staging/state tiles
    xpool = P.tile([128, YR * WP], BF16, tag="xpool")   # later reused as oa
    plh = P.tile([128, YR, 48], BF16, tag="plh")
    pl = P.tile([128, 26, 48], BF16, tag="pl")
    vint = P.tile([128, TR, 48], BF16, tag="vint")
    tscr = P.tile([128, TR, 48], BF16, tag="tscr")
    brs = P.tile([128, 51 * WP], BF16, tag="brs")       # later reused as ob
    Ts = [P.tile([T_SPECS[i], TR * WP], BF16, tag=f"T{i}", name=f"T{i}")
          for i in range(5)]
    qk = [P.tile([128, SR * W], BF16, tag=f"qk{i}", name=f"qk{i}")
          for i in range(3)]
    v0 = P.tile([128, QR * W], BF16, tag="v0")
    v1 = P.tile([64, QR * W], BF16, tag="v1")
    sqac = [P.tile([128, 10], F32, tag=f"sqac{i}", name=f"sqac{i}")
            for i in range(3)]
    accs = [P.tile([128, 1], F32, tag=f"accs{i}", name=f"accs{i}")
            for i in range(3)]
    ssa = P.tile([128, 194], F32, tag="ssa")
    ssb = P.tile([64, 194], F32, tag="ssb")
    sfa = P.tile([128, 194], F32, tag="sfa")
    sfb = P.tile([64, 194], F32, tag="sfb")
    rska = P.tile([128, 1], F32, tag="rska")
    rskb = P.tile([64, 1], F32, tag="rskb")
    rqa = P.tile([128, 1], F32, tag="rqa")
    rqb = P.tile([64, 1], F32, tag="rqb")
    sta = P.tile([128, C], F32, tag="sta")
    stb = P.tile([64, C], F32, tag="stb")
    paf = P.tile([128, C], BF16, tag="paf")
    pbf = P.tile([64, C], BF16, tag="pbf")
    pta = P.tile([128, C], BF16, tag="pta")
    ptb = P.tile([64, C], BF16, tag="ptb")

    xfv = xfold.rearrange("p (r w) -> p r w", w=WP)
    xpv = xpool.rearrange("p (r w) -> p r w", w=WP)
    brv = brs.rearrange("p (r w) -> p r w", w=WP)
    Tv = [t.rearrange("p (r w) -> p r w", w=WP) for t in Ts]
    qkv = [t.rearrange("p (r w) -> p r w", w=W) for t in qk]
    v0v = v0.rearrange("p (r w) -> p r w", w=W)
    v1v = v1.rearrange("p (r w) -> p r w", w=W)
    w0fv = w0f.rearrange("p (t m) -> p t m", t=3)
    w0gv = w0g.rearrange("p (t m) -> p t m", t=3)
    w1v = [w.rearrange("p (t m) -> p t m", t=3) for w in w1s]
    w2av = w2da.rearrange("p (t m) -> p t m", t=9)
    w2fv = w2fb.rearrange("p (t m) -> p t m", t=3)
    w2gv = w2gb.rearrange("p (t m) -> p t m", t=3)

    # border memsets: col 0 of shifted x copy; T tiles row0 + cols 0/97;
    # brs row 0 + cols 0/97 (brs interior rows fully rewritten)
    nc.vector.memset(xfv[0:64, :, 0:1], 0.0)
    for i in range(5):
        p = T_SPECS[i]
        nc.vector.memset(Tv[i][0:p, 0:1, :], 0.0)
        nc.vector.memset(Tv[i][0:p, :, 0:1], 0.0)
        nc.vector.memset(Tv[i][0:p, :, 97:98], 0.0)
    nc.vector.memset(brv[:, 0:1, :], 0.0)
    nc.vector.memset(brv[:, :, 0:1], 0.0)
    nc.vector.memset(brv[:, :, 97:98], 0.0)

    # ---------------- conv0 ----------------
    # out-tile A (128): x1|x3 -> xpool; out-tile B (64): x2 -> T slots
    blocksA = _row_blocks(YR)
    blocksB = _row_blocks(TR)
    with tc.tile_pool(name="ps_c0", bufs=4, space="PSUM") as pp0:
        for (r0, nr) in blocksA:
            ps = pp0.tile([128, 5, W], F32, tag="c0psA")
            for dy in range(3):
                nc.tensor.matmul(
                    ps[:, 0:nr, :], lhsT=w0fv[:, dy, 0:128],
                    rhs=xfv[:, r0 + dy:r0 + dy + nr, 1:97],
                    start=(dy == 0), stop=False)
                nc.tensor.matmul(
                    ps[:, 0:nr, :], lhsT=w0gv[64:128, dy, 0:128],
                    rhs=xfv[64:128, r0 + dy:r0 + dy + nr, 2:98],
                    start=False, stop=(dy == 2))
            nc.scalar.activation(
                out=xpv[:, r0:r0 + nr, 1:97], in_=ps[:, 0:nr, :],
                func=mybir.ActivationFunctionType.Relu,
                bias=sc0a[:, 1:2], scale=sc0a[:, 0:1])
        for (r0, nr) in blocksB:
            ps = pp0.tile([128, 5, W], F32, tag="c0psB")
            for dy in range(3):
                nc.tensor.matmul(
                    ps[0:64, 0:nr, :], lhsT=w0fv[:, dy, 128:192],
                    rhs=xfv[:, r0 + dy:r0 + dy + nr, 1:97],
                    start=(dy == 0), stop=False)
                nc.tensor.matmul(
                    ps[0:64, 0:nr, :], lhsT=w0gv[64:128, dy, 128:192],
                    rhs=xfv[64:128, r0 + dy:r0 + dy + nr, 2:98],
                    start=False, stop=(dy == 2))
            # x2 = t channels 64..127 -> ky1 slot T2[0:64]; the ky0/ky2 slots
            # (T0[64:]/T3[64:]) are bulk row-shifted DMA copies of it below
            nc.scalar.activation(
                out=Tv[2][0:64, r0:r0 + nr, 1:97], in_=ps[0:64, 0:nr, :],
                func=mybir.ActivationFunctionType.Relu,
                bias=sc0b[:, 1:2], scale=sc0b[:, 0:1])

    # x2 ky-shifted slot copies (chunked; T2 col pads are zero so full width)
    for (a, b) in ((0, 30), (30, 49)):
        nc.sync.dma_start(out=Tv[0][64:128, a + 1:b + 1, :],
                          in_=Tv[2][0:64, a:b, :])
        nc.sync.dma_start(out=Tv[3][64:128, a:b, :],
                          in_=Tv[2][0:64, a + 1:b + 1, :])

    # ---------------- pools + bilinear (chunked over pl rows) ----------------
    cA = P.tile([128, 2], F32, tag="cA")
    nc.vector.memset(cA[0:64, 0:1], 0.75)
    nc.vector.memset(cA[0:64, 1:2], 0.25)
    nc.vector.memset(cA[64:128, 0:1], 0.1875)
    nc.vector.memset(cA[64:128, 1:2], 0.0625)
    cC = P.tile([128, 1], F32, tag="cC")
    nc.vector.memset(cC[0:64, :], 1.0)
    nc.vector.memset(cC[64:128, :], 0.25)

    pl_chunks = [(0, 5), (5, 10), (10, 15), (15, 20), (20, 25), (25, 26)]
    kv_o = 0   # next odd-row k (vint[2k+1], k<=24)
    kv_e = 0   # next even-row k (vint[2k+2], k<=23)
    hv = 0     # next t-row for the horizontal pass

    def hpass(a, b):
        if b <= a:
            return
        nc.vector.tensor_scalar(out=brv[:, 1 + a:1 + b, 1:2],
                                in0=vint[:, a:b, 0:1], scalar1=cC[:, 0:1],
                                scalar2=None, op0=mybir.AluOpType.mult)
        nc.vector.tensor_scalar(out=brv[:, 1 + a:1 + b, 96:97],
                                in0=vint[:, a:b, 47:48], scalar1=cC[:, 0:1],
                                scalar2=None, op0=mybir.AluOpType.mult)
        nc.vector.tensor_scalar(out=tscr[:, a:b, 0:47], in0=vint[:, a:b, 1:48],
                                scalar1=cA[:, 1:2], scalar2=None,
                                op0=mybir.AluOpType.mult)
        nc.vector.scalar_tensor_tensor(
            out=brv[:, 1 + a:1 + b, 2:96:2], in0=vint[:, a:b, 0:47],
            scalar=cA[:, 0:1], in1=tscr[:, a:b, 0:47],
            op0=mybir.AluOpType.mult, op1=mybir.AluOpType.add)
        nc.vector.tensor_scalar(out=tscr[:, a:b, 0:47], in0=vint[:, a:b, 1:48],
                                scalar1=cA[:, 0:1], scalar2=None,
                                op0=mybir.AluOpType.mult)
        nc.vector.scalar_tensor_tensor(
            out=brv[:, 1 + a:1 + b, 3:96:2], in0=vint[:, a:b, 0:47],
            scalar=cA[:, 1:2], in1=tscr[:, a:b, 0:47],
            op0=mybir.AluOpType.mult, op1=mybir.AluOpType.add)

    for (k0, k1) in pl_chunks:
        # horizontal pool pairs for y rows 2k0..2k1-1
        nc.vector.tensor_tensor(out=plh[0:64, 2 * k0:2 * k1, :],
                                in0=xpv[0:64, 2 * k0:2 * k1, 1:97:2],
                                in1=xpv[0:64, 2 * k0:2 * k1, 2:98:2],
                                op=mybir.AluOpType.max)
        nc.vector.tensor_tensor(out=plh[64:128, 2 * k0:2 * k1, :],
                                in0=xpv[64:128, 2 * k0:2 * k1, 1:97:2],
                                in1=xpv[64:128, 2 * k0:2 * k1, 2:98:2],
                                op=mybir.AluOpType.add)
        # vertical pool pairs -> pl rows k0..k1-1
        nc.vector.tensor_tensor(out=pl[0:64, k0:k1, :],
                                in0=plh[0:64, 2 * k0:2 * k1:2, :],
                                in1=plh[0:64, 2 * k0 + 1:2 * k1:2, :],
                                op=mybir.AluOpType.max)
        nc.vector.tensor_tensor(out=pl[64:128, k0:k1, :],
                                in0=plh[64:128, 2 * k0:2 * k1:2, :],
                                in1=plh[64:128, 2 * k0 + 1:2 * k1:2, :],
                                op=mybir.AluOpType.add)
        if k0 == 0:
            nc.vector.tensor_copy(out=vint[:, 0, :], in_=pl[:, 0, :])
        # vertical bilinear rows that only need pl rows < k1
        ke_o = min(k1 - 1, 25)
        if ke_o > kv_o:
            a, b = kv_o, ke_o
            nc.vector.tensor_scalar(out=tscr[:, a:b, :], in0=pl[:, a + 1:b + 1, :],
                                    scalar1=0.25, scalar2=None,
                                    op0=mybir.AluOpType.mult)
            nc.vector.scalar_tensor_tensor(
                out=vint[:, 2 * a + 1:2 * b:2, :], in0=pl[:, a:b, :],
                scalar=0.75, in1=tscr[:, a:b, :],
                op0=mybir.AluOpType.mult, op1=mybir.AluOpType.add)
            kv_o = ke_o
        ke_e = min(k1 - 1, 24)
        if ke_e > kv_e:
            a, b = kv_e, ke_e
            nc.vector.tensor_scalar(out=tscr[:, a:b, :], in0=pl[:, a + 1:b + 1, :],
                                    scalar1=0.75, scalar2=None,
                                    op0=mybir.AluOpType.mult)
            nc.vector.scalar_tensor_tensor(
                out=vint[:, 2 * a + 2:2 * b + 1:2, :], in0=pl[:, a:b, :],
                scalar=0.25, in1=tscr[:, a:b, :],
                op0=mybir.AluOpType.mult, op1=mybir.AluOpType.add)
            kv_e = ke_e
        # horizontal pass over fully-available vint rows
        avail = min(2 * kv_o + 1, 2 * kv_e + 2) if k1 < 26 else TR
        hpass(hv, avail)
        hv = avail

    # brs rows 0..50 (row 0 zero, rows 1..50 = t rows 0..49) -> T slots
    # (ky slot s stores t row rho at tile row rho+1-s). Row-chunked so
    # conv1's early blocks start on the first half.
    for (a, b) in ((0, 10), (10, 30), (30, 50)):
        nc.sync.dma_start(out=Tv[0][0:64, a:b, :], in_=brv[0:64, a:b, :])
        nc.sync.dma_start(out=Tv[1][0:64, a:b, :], in_=brv[64:128, a:b, :])
        nc.sync.dma_start(out=Tv[1][64:128, a:b, :],
                          in_=brv[0:64, a + 1:b + 1, :])
        nc.sync.dma_start(out=Tv[2][64:128, a:b, :],
                          in_=brv[64:128, a + 1:b + 1, :])
        bb = min(b, 49)
        nc.sync.dma_start(out=Tv[3][0:64, a:bb, :],
                          in_=brv[0:64, a + 2:bb + 2, :])
        nc.sync.dma_start(out=Tv[4][0:64, a:bb, :],
                          in_=brv[64:128, a + 2:bb + 2, :])

    # oa (= xpool reuse) and ob (= brs reuse, kx-folded: p0..63 = ob shifted
    # right one col, p64..127 = ob) borders for the depthwise reads
    oav = xpv[:, 0:50, :]
    nc.vector.memset(oav[:, 0:1, :], 0.0)
    nc.vector.memset(oav[:, :, 0:1], 0.0)
    nc.vector.memset(oav[:, :, 97:98], 0.0)
    obv = brv[:, 0:50, :]
    nc.vector.memset(obv[0:64, :, 1:2], 0.0)

    # ---------------- conv1 + attention prologue ----------------
    qk_blocks = _row_blocks(SR)
    v_blocks = _row_blocks(QR)

    def conv1_tile(ot, blocks, dst, dstv, sq):
        mw = T_SPECS[ot] if ot == 4 else 128
        m0 = 128 * ot
        for bi, (r0, nr) in enumerate(blocks):
            ps = pp1.tile([128, 5, W], F32, tag="c1ps")
            first = True
            for ti in range(5):
                tp = T_SPECS[ti]
                for dx in range(3):
                    nc.tensor.matmul(
                        ps[0:mw, 0:nr, :],
                        lhsT=w1v[ti][:, dx, m0:m0 + mw],
                        rhs=Tv[ti][0:tp, r0:r0 + nr, dx:dx + 96],
                        start=first, stop=(ti == 4 and dx == 2))
                    first = False
            nc.scalar.activation(
                out=dstv[0:mw, r0:r0 + nr, :], in_=ps[0:mw, 0:nr, :],
                func=mybir.ActivationFunctionType.Relu,
                bias=sc1[ot][:, 1:2], scale=sc1[ot][:, 0:1])
            if sq is not None:
                dump = ev.tile([128, 5 * W], BF16, tag="sqd")
                nc.scalar.activation(
                    out=dump[:, 0:nr * W],
                    in_=dst[:, r0 * W:(r0 + nr) * W],
                    func=mybir.ActivationFunctionType.Square,
                    accum_out=sq[:, bi:bi + 1])

    with tc.tile_pool(name="ps_c1", bufs=3, space="PSUM") as pp1, \
         tc.tile_pool(name="ps_tr", bufs=2, space="PSUM") as ppt, \
         tc.tile_pool(name="ps_s", bufs=1, space="PSUM") as pps:

        for ot in range(3):
            conv1_tile(ot, qk_blocks, qk[ot], qkv[ot], sqac[ot])

        # transposes + unnormalized S'^T partial (rows = k channels)
        qk0r = qk[0].rearrange("p (c k) -> p c k", k=128)
        qk1r = qk[1].rearrange("p (c k) -> p c k", k=128)
        qk2r = qk[2].rearrange("p (c k) -> p c k", k=128)
        sp = pps.tile([128, 2 * C], F32, tag="sp")
        for g in range(NCH // 3):
            tq = ppt.tile([128, 3 * C], BF16, tag="tq")
            tk = ppt.tile([128, 3 * C], BF16, tag="tk")
            tqv = tq.rearrange("p (i c) -> p i c", c=C)
            tkv = tk.rearrange("p (i c) -> p i c", c=C)
            for i in range(3):
                ci = 3 * g + i
                nc.tensor.transpose(tqv[:, i, 0:128], qk0r[:, ci, :], ident[:])
                nc.tensor.transpose(tqv[:, i, 128:192], qk1r[0:64, ci, :],
                                    ident[0:64, 0:64])
                nc.tensor.transpose(tkv[:, i, 0:64], qk1r[64:128, ci, :],
                                    ident[64:128, 64:128])
                nc.tensor.transpose(tkv[:, i, 64:192], qk2r[:, ci, :], ident[:])
            qtc = ev.tile([128, 3 * C], BF16, tag="qtc")
            ktc = ev.tile([128, 3 * C], BF16, tag="ktc")
            nc.scalar.copy(out=qtc[:], in_=tq[:])
            nc.scalar.copy(out=ktc[:], in_=tk[:])
            qcv = qtc.rearrange("p (i c) -> p i c", c=C)
            kcv = ktc.rearrange("p (i c) -> p i c", c=C)
            for i in range(3):
                nc.tensor.matmul(sp[:, 0:C], lhsT=kcv[:, i, 0:128],
                                 rhs=qcv[:, i, :],
                                 start=(g == 0 and i == 0),
                                 stop=(g == NCH // 3 - 1 and i == 2))
                nc.tensor.matmul(sp[0:64, C:2 * C], lhsT=kcv[:, i, 128:192],
                                 rhs=qcv[:, i, :],
                                 start=(g == 0 and i == 0),
                                 stop=(g == NCH // 3 - 1 and i == 2))

        # sumsq totals + AllReduce staging
        for ti in range(3):
            nc.vector.reduce_sum(out=accs[ti][:], in_=sqac[ti][:],
                                 axis=mybir.AxisListType.X)
        nc.vector.tensor_copy(out=ssa[:, 192:193], in_=accs[0][:])
        nc.vector.tensor_copy(out=ssb[:, 192:193], in_=accs[1][0:64, :])
        nc.sync.dma_start(out=ssa[0:64, 193:194], in_=accs[1][64:128, :])
        nc.sync.dma_start(out=ssa[64:128, 193:194], in_=accs[2][0:64, :])
        nc.sync.dma_start(out=ssb[0:64, 193:194], in_=accs[2][64:128, :])
        nc.scalar.copy(out=ssa[:, 0:192], in_=sp[:, 0:C])
        nc.scalar.copy(out=ssb[:, 0:192], in_=sp[0:64, C:2 * C])
        nc.gpsimd.dma_start(out=cc2i[0:128, :], in_=ssa[:])
        nc.gpsimd.dma_start(out=cc2i[128:192, :], in_=ssb[:])
        nc.gpsimd.collective_compute(
            "AllReduce", mybir.AluOpType.add, replica_groups=GROUPS,
            ins=[cc2i[:]], outs=[cc2o[:]])
        nc.gpsimd.dma_start(out=sfa[:], in_=cc2o[0:128, :])
        nc.gpsimd.dma_start(out=sfb[:], in_=cc2o[128:192, :])

        # conv1 v tiles overlap the collective
        conv1_tile(3, v_blocks, v0, v0v, None)
        conv1_tile(4, v_blocks, v1, v1v, None)

    # ---------------- normalize logits + softmax + P^T ----------------
    for (sf, rsk, rq, st, p) in ((sfa, rska, rqa, sta, 128),
                                 (sfb, rskb, rqb, stb, 64)):
        nc.scalar.activation(out=rsk[:], in_=sf[0:p, 193:194],
                             func=mybir.ActivationFunctionType.Sqrt)
        nc.scalar.activation(out=rq[:], in_=sf[0:p, 192:193],
                             func=mybir.ActivationFunctionType.Sqrt)
        for r in (rsk, rq):
            nc.vector.tensor_scalar(out=r[:], in0=r[:], scalar1=1e-12,
                                    scalar2=None, op0=mybir.AluOpType.max)
            nc.vector.reciprocal(out=r[:], in_=r[:])
        nc.vector.tensor_tensor(out=rq[:], in0=rq[:], in1=tmps[0:p, :],
                                op=mybir.AluOpType.mult)
        # scale S'^T rows (k channels) by 1/|k|
        nc.vector.tensor_scalar(out=st[:], in0=sf[0:p, 0:192], scalar1=rsk[:],
                                scalar2=None, op0=mybir.AluOpType.mult)

    with tc.tile_pool(name="ps_pt", bufs=1, space="PSUM") as ppm, \
         tc.tile_pool(name="ps_pv", bufs=2, space="PSUM") as ppv:
        sps1 = ppm.tile([128, C], F32, tag="sps1")
        nc.tensor.transpose(sps1[:, 0:128], sta[:, 0:128], identf[:])
        nc.tensor.transpose(sps1[:, 128:192], stb[:, 0:128],
                            identf[0:64, 0:64])
        sps2 = ppm.tile([64, C], F32, tag="sps2")
        nc.tensor.transpose(sps2[0:64, 0:128], sta[:, 128:192], identf[:])
        nc.tensor.transpose(sps2[0:64, 128:192], stb[:, 128:192],
                            identf[0:64, 0:64])

        for (sps, rq, pf, p) in ((sps1, rqa, paf, 128), (sps2, rqb, pbf, 64)):
            mx = ev.tile([128, 1], F32, tag="mx")
            nb = ev.tile([128, 1], F32, tag="nb")
            sm = ev.tile([128, 1], F32, tag="sm")
            pexp = ev.tile([128, C], F32, tag="pexp")
            nc.vector.reduce_max(out=mx[0:p, :], in_=sps[0:p, :],
                                 axis=mybir.AxisListType.X)
            nc.vector.scalar_tensor_tensor(
                out=nb[0:p, :], in0=mx[0:p, :], scalar=-1.0, in1=rq[:],
                op0=mybir.AluOpType.mult, op1=mybir.AluOpType.mult)
            nc.scalar.activation(out=pexp[0:p, :], in_=sps[0:p, :],
                                 func=mybir.ActivationFunctionType.Exp,
                                 bias=nb[0:p, :], scale=rq[:],
                                 accum_out=sm[0:p, :])
            nc.vector.reciprocal(out=sm[0:p, :], in_=sm[0:p, :])
            nc.vector.tensor_scalar(out=pf[:], in0=pexp[0:p, :],
                                    scalar1=sm[0:p, :], scalar2=None,
                                    op0=mybir.AluOpType.mult)

        tp1 = ppm.tile([128, C], BF16, tag="tp1")
        nc.tensor.transpose(tp1[:, 0:128], paf[:, 0:128], ident[:])
        nc.tensor.transpose(tp1[:, 128:192], pbf[:, 0:128], ident[0:64, 0:64])
        nc.scalar.copy(out=pta[:], in_=tp1[:])
        tp2 = ppm.tile([64, C], BF16, tag="tp2")
        nc.tensor.transpose(tp2[0:64, 0:128], paf[:, 128:192], ident[:])
        nc.tensor.transpose(tp2[0:64, 128:192], pbf[:, 128:192],
                            ident[0:64, 0:64])
        nc.scalar.copy(out=ptb[:], in_=tp2[0:64, :])

        # out = P @ v
        for (r0, nr) in v_blocks:
            po = ppv.tile([128, 5, W], F32, tag="po")
            po2 = ppv.tile([128, 5, W], F32, tag="po2")
            nc.tensor.matmul(po[:, 0:nr, :], lhsT=pta[:, 0:128],
                             rhs=v0v[:, r0:r0 + nr, :], start=True, stop=False)
            nc.tensor.matmul(po[:, 0:nr, :], lhsT=ptb[:, 0:128],
                             rhs=v1v[:, r0:r0 + nr, :], start=False, stop=True)
            nc.tensor.matmul(po2[0:64, 0:nr, :], lhsT=pta[:, 128:192],
                             rhs=v0v[:, r0:r0 + nr, :], start=True, stop=False)
            nc.tensor.matmul(po2[0:64, 0:nr, :], lhsT=ptb[:, 128:192],
                             rhs=v1v[:, r0:r0 + nr, :], start=False, stop=True)
            nc.scalar.copy(out=oav[:, r0 + 1:r0 + 1 + nr, 1:97],
                           in_=po[:, 0:nr, :])
            nc.scalar.copy(out=obv[0:64, r0 + 1:r0 + 1 + nr, 2:98],
                           in_=po2[0:64, 0:nr, :])
        # replicate ob to partitions 64..127 at the unshifted column offset
        nc.sync.dma_start(out=obv[64:128, 1:50, 1:97],
                          in_=obv[0:64, 1:50, 2:98])

    # ---------------- depthwise conv + bias ----------------
    yv = yout.rearrange("c (r w) -> c r w", w=W)
    with tc.tile_pool(name="ps_dw", bufs=4, space="PSUM") as ppd:
        for (r0, nr) in _row_blocks(OR_):
            ps = ppd.tile([128, 5, W], F32, tag="dwps")
            for t in range(9):
                dy, dx = t // 3 - 1, t % 3 - 1
                nc.tensor.matmul(
                    ps[:, 0:nr, :],
                    lhsT=w2av[:, t, :],
                    rhs=oav[:, r0 + 1 + dy:r0 + 1 + dy + nr, 1 + dx:97 + dx],
                    start=(t == 0), stop=(t == 8))
            fo = ev.tile([128, 5, W], F32, tag="fo")
            nc.scalar.activation(out=fo[:, 0:nr, :], in_=ps[:, 0:nr, :],
                                 func=mybir.ActivationFunctionType.Identity,
                                 bias=b2a[:, 0:1], scale=1.0)
            nc.sync.dma_start(out=yv[0:128, r0:r0 + nr, :],
                              in_=fo[:, 0:nr, :])
        for (r0, nr) in _row_blocks(OR_):
            ps = ppd.tile([128, 5, W], F32, tag="dwps")
            for dy in range(3):
                nc.tensor.matmul(
                    ps[0:64, 0:nr, :], lhsT=w2fv[:, dy, :],
                    rhs=obv[:, r0 + dy:r0 + dy + nr, 1:97],
                    start=(dy == 0), stop=False)
                nc.tensor.matmul(
                    ps[0:64, 0:nr, :], lhsT=w2gv[64:128, dy, :],
                    rhs=obv[64:128, r0 + dy:r0 + dy + nr, 2:98],
                    start=False, stop=(dy == 2))
            fo = ev.tile([128, 5, W], F32, tag="fo")
            nc.scalar.activation(out=fo[0:64, 0:nr, :], in_=ps[0:64, 0:nr, :],
                                 func=mybir.ActivationFunctionType.Identity,
                                 bias=b2b[:, 0:1], scale=1.0)
            nc.sync.dma_start(out=yv[128:192, r0:r0 + nr, :],
                              in_=fo[0:64, 0:nr, :])
    ctx.close()


# ---------------- host side ----------------
_NC_CACHE = None


def _get_nc():
    global _NC_CACHE
    if _NC_CACHE is None:
        _NC_CACHE = build_nc()
    return _NC_CACHE


def _pack_weights(inp, flip):
    bf = ml_dtypes.bfloat16
    w0 = inp["w0"][:, :, ::-1, :] if flip else inp["w0"]
    w1 = inp["w1"][:, :, ::-1, :] if flip else inp["w1"]
    w2 = inp["w2"][:, :, ::-1, :] if flip else inp["w2"]
    w0 = np.asarray(w0, np.float32)
    w1 = np.asarray(w1, np.float32)
    w2 = np.asarray(w2, np.float32)

    # conv0: out-channel order [x1(0:64), x3(128:192), x2(64:128)]
    cho = np.concatenate([np.arange(0, 64), np.arange(128, 192),
                          np.arange(64, 128)])
    w0p = w0[cho]                       # [192, 64, 3, 3]
    w0f = np.zeros((128, 3, C), np.float32)
    w0f[0:64] = w0p[:, :, :, 0].transpose(1, 2, 0)    # (c, kx=0)
    w0f[64:128] = w0p[:, :, :, 1].transpose(1, 2, 0)  # (c, kx=1)
    w0g = np.zeros((128, 3, C), np.float32)           # data at partitions 64..
    w0g[64:128] = w0p[:, :, :, 2].transpose(1, 2, 0)
    s0 = inp["g0"] / np.sqrt(inp["v0"] + BN_EPS)
    t0 = inp["be0"] + (inp["b0"] - inp["m0"]) * s0
    sb0 = np.stack([s0, t0], axis=1).astype(np.float32)[cho]

    # conv1: contraction partition P = ky*192 + c, tiles of 128
    w1tiles = []
    bounds = [0, 128, 256, 384, 512, 576]
    for i in range(5):
        Pr = np.arange(bounds[i], bounds[i + 1])
        cc, ky = Pr % C, Pr // C
        wt = w1[:, cc, ky, :].transpose(1, 2, 0)   # [np, 3, 576]
        w1tiles.append(np.ascontiguousarray(wt.reshape(len(Pr), 3 * C3)))
    s1 = inp["g1"] / np.sqrt(inp["v1"] + BN_EPS)
    t1 = inp["be1"] + (inp["b1"] - inp["m1"]) * s1
    sb1 = np.stack([s1, t1], axis=1).astype(np.float32)

    w2da = np.zeros((128, 9, 128), np.float32)
    w2fb = np.zeros((128, 3, 64), np.float32)
    w2gb = np.zeros((128, 3, 64), np.float32)
    r64, r128 = np.arange(64), np.arange(128)
    for t in range(9):
        d = w2[:, 0, t // 3, t % 3]
        w2da[r128, t, r128] = d[0:128]
    for dy in range(3):
        db = w2[128:192, 0, dy, :]
        w2fb[r64, dy, r64] = db[:, 0]
        w2fb[64 + r64, dy, r64] = db[:, 1]
        w2gb[64 + r64, dy, r64] = db[:, 2]

    out = {
        "w0f": np.ascontiguousarray(w0f.reshape(128, 3 * C)).astype(bf),
        "w0g": np.ascontiguousarray(w0g.reshape(128, 3 * C)).astype(bf),
        "sb0p": sb0,
        "sb1": sb1,
        "w2da": np.ascontiguousarray(w2da.reshape(128, 9 * 128)).astype(bf),
        "w2fb": np.ascontiguousarray(w2fb.reshape(128, 3 * 64)).astype(bf),
        "w2gb": np.ascontiguousarray(w2gb.reshape(128, 3 * 64)).astype(bf),
        "b2v": np.asarray(inp["b2"], np.float32).reshape(C, 1),
    }
    for i in range(5):
        out[f"w1t{i}"] = w1tiles[i].astype(bf)
    return out


def kernel(**inputs):
    inputs = {k: np.asarray(v) for k, v in inputs.items()}
    x = inputs["x"]
    B = x.shape[0]
    bf = ml_dtypes.bfloat16
    packs = [_pack_weights(inputs, flip) for flip in (False, True)]
    tempv = np.asarray(inputs["temp"], np.float32).reshape(1, 1)

    in_maps = []
    for core in range(8):
        s, h = core // 2, core % 2
        xi = np.asarray(x[s], np.float32)
        if h:
            xi = xi[:, ::-1, :]
        slab = np.zeros((64, XR, WP), np.float32)
        slab[:, 1:54, 1:97] = xi[:, 0:53, :]
        m = dict(packs[h])
        m["xs"] = np.ascontiguousarray(slab.reshape(64, XR * WP)).astype(bf)
        m["tempv"] = tempv
        in_maps.append(m)

    nc = _get_nc()
    res = run_bass_kernel_spmd(nc, in_maps, list(range(8)))
    out = np.zeros((B, C, 96, 96), np.float32)
    for core in range(8):
        s, h = core // 2, core % 2
        yc = res.results[core]["yout"].reshape(C, OR_, W)
        if h:
            out[s, :, 48:96] = yc[:, ::-1, :]
        else:
            out[s, :, 0:48] = yc
    return out


# revision 6
# speedup vs baseline: 1.0066x; 1.0066x over previous
"""Bass/Trainium2 kernel for nn_CSEM sparse_attention problem (v2).

Sharding: 8 cores = 4 samples x 2 spatial halves (bottom half vertically
flipped on host so all cores run an identical NEFF).

v2 vs baseline:
- conv0: (cin, kx{0,1}) folded into a 128-partition contraction tile; x1|x3
  merged into one 128-wide output tile (12 -> ~6 matmuls/block).
- conv1: (c, ky) folded into 5 contraction tiles T0..T4 (row-shifted copies
  of t), 128-aligned output tiles -> 15 full matmuls/block-tile vs 18.
- single AllReduce: S' = q'k'^T unnormalized + sumsq(q/k) as 2 extra f32
  columns; normalization applied to the 192x192 logits post-collective.
  The collective overlaps conv1's v output tiles.
- sumsq on Act engine (Square + accum_out), pools/bilinear chunked to
  overlap conv0, DMAs issued from the idle SP engine, border-only memsets.
"""

import numpy as np
import ml_dtypes

import concourse.bass as bass
import concourse.mybir as mybir
import concourse.tile as tile
from concourse.bass_utils import run_bass_kernel_spmd
from concourse.masks import make_identity

BF16 = mybir.dt.bfloat16
F32 = mybir.dt.float32
BN_EPS = 1e-5

CIN, C, C3 = 64, 192, 576
W, WP = 96, 98
XR = 54          # x slab rows (1 zero pad + 53 data)
YR = 52          # y rows computed locally (0..51)
TR = 50          # t rows 0..49
QR = 49          # conv1 v rows 0..48
SR = 48          # rows feeding S partial
OR_ = 48         # final output rows per core
NCH = SR * W // 128   # 36 transpose chunks
GROUPS = [[0, 1], [2, 3], [4, 5], [6, 7]]

# conv1 contraction tiles: partition P = ky*192 + c, tiles of 128
#   T0: ky0 c0..127 | T1: ky0 c128..191 + ky1 c0..63 | T2: ky1 c64..191
#   T3: ky2 c0..127 | T4: ky2 c128..191 (64 partitions)
T_SPECS = [128, 128, 128, 128, 64]


def _split_waits(nc, limit=1):
    """This walrus build rejects instructions carrying more than one sem-wait
    command. Spread extra waits onto same-engine ENGINE_NOPs inserted just
    before the offending instruction (semantically identical: the engine
    blocks on each wait in program order)."""
    ctr = [0]
    for f in nc.m.functions:
        for blk in f.blocks:
            il = blk.instructions
            new = []
            for inst in il:
                si = inst.sync_info
                waits = list(si.on_wait) if (si and si.on_wait) else []
                if len(waits) > limit:
                    for w in waits[:-limit]:
                        ctr[0] += 1
                        nop = mybir.InstNoOp(name=f"WNOP-{ctr[0]}")
                        nop.engine = inst.engine
                        nop.sync_info = mybir.SyncInfo(on_wait=[w], on_update=[])
                        new.append(nop)
                    si.on_wait = waits[-limit:]
                new.append(inst)
            il[:] = new


def _row_blocks(nrows, per=5):
    out, r = [], 0
    while r < nrows:
        n = min(per, nrows - r)
        out.append((r, n))
        r += n
    return out


def build_nc():
    nc = bass.Bass()

    xs_d = nc.declare_dram_parameter("xs", [64, XR * WP], BF16, isOutput=False)
    w0f_d = nc.declare_dram_parameter("w0f", [128, 3 * C], BF16, isOutput=False)
    w0g_d = nc.declare_dram_parameter("w0g", [128, 3 * C], BF16, isOutput=False)
    sb0_d = nc.declare_dram_parameter("sb0p", [C, 2], F32, isOutput=False)
    w1_d = [nc.declare_dram_parameter(f"w1t{i}", [T_SPECS[i], 3 * C3], BF16,
                                      isOutput=False) for i in range(5)]
    sb1_d = nc.declare_dram_parameter("sb1", [C3, 2], F32, isOutput=False)
    w2a_d = nc.declare_dram_parameter("w2da", [128, 9 * 128], BF16, isOutput=False)
    w2f_d = nc.declare_dram_parameter("w2fb", [128, 3 * 64], BF16, isOutput=False)
    w2g_d = nc.declare_dram_parameter("w2gb", [128, 3 * 64], BF16, isOutput=False)
    b2_d = nc.declare_dram_parameter("b2v", [C, 1], F32, isOutput=False)
    tmp_d = nc.declare_dram_parameter("tempv", [1, 1], F32, isOutput=False)
    yout = nc.declare_dram_parameter("yout", [C, OR_ * W], F32, isOutput=True)

    cc2i = nc.dram_tensor("cc2i", [C, 194], F32)
    cc2o = nc.dram_tensor("cc2o", [C, 194], F32)

    with tile.TileContext(nc) as tc:
        _body(nc, tc, xs_d, w0f_d, w0g_d, sb0_d, w1_d, sb1_d, w2a_d, w2f_d,
              w2g_d, b2_d, tmp_d, yout, cc2i, cc2o)
    _split_waits(nc)
    return nc


def _body(nc, tc, xs_d, w0f_d, w0g_d, sb0_d, w1_d, sb1_d, w2a_d, w2f_d,
          w2g_d, b2_d, tmp_d, yout, cc2i, cc2o):
    import contextlib
    ctx = contextlib.ExitStack()
    P = ctx.enter_context(tc.tile_pool(name="persist", bufs=1))
    ev = ctx.enter_context(tc.tile_pool(name="evac", bufs=3))

    # ---- persistent SBUF ----
    xfold = P.tile([128, XR * WP], BF16, tag="xfold")
    w0f = P.tile([128, 3 * C], BF16, tag="w0f")
    w0g = P.tile([128, 3 * C], BF16, tag="w0g")  # data in partitions 64..127
    w1s = [P.tile([T_SPECS[i], 3 * C3], BF16, tag=f"w1s{i}", name=f"w1s{i}")
           for i in range(5)]
    w2da = P.tile([128, 9 * 128], BF16, tag="w2da")
    w2fb = P.tile([128, 3 * 64], BF16, tag="w2fb")
    w2gb = P.tile([128, 3 * 64], BF16, tag="w2gb")  # data at partitions 64..
    sc0a = P.tile([128, 2], F32, tag="sc0a")
    sc0b = P.tile([64, 2], F32, tag="sc0b")
    scp = [128, 128, 128, 128, 64]
    sc1 = [P.tile([scp[i], 2], F32, tag=f"sc1_{i}", name=f"sc1_{i}")
           for i in range(5)]
    b2a = P.tile([128, 1], F32, tag="b2a")
    b2b = P.tile([64, 1], F32, tag="b2b")
    tmps = P.tile([128, 1], F32, tag="tmps")

    # x fold: p0..63 = x shifted right one col (kx=0 operand), p64..127 = x.
    # Row-chunked so conv0 starts on the first half.
    xfv0 = xfold.rearrange("p (r w) -> p r w", w=WP)
    nc.sync.dma_start(out=w0f[:], in_=w0f_d[:])
    nc.sync.dma_start(out=w0g[:], in_=w0g_d[:])
    for (a, b) in ((0, 8), (8, 32), (32, XR)):
        nc.sync.dma_start(out=xfold[64:128, a * WP:b * WP],
                          in_=xs_d[:, a * WP:b * WP])
        nc.sync.dma_start(
            out=xfv0[0:64, a:b, 1:98],
            in_=bass.AP(tensor=xs_d, offset=a * WP,
                        ap=[[XR * WP, 64], [WP, b - a], [1, 97]]))
    for i in range(5):
        nc.sync.dma_start(out=w1s[i][:], in_=w1_d[i][:])
    nc.sync.dma_start(out=w2da[:], in_=w2a_d[:])
    nc.sync.dma_start(out=w2fb[:], in_=w2f_d[:])
    nc.sync.dma_start(out=w2gb[:], in_=w2g_d[:])
    nc.sync.dma_start(out=sc0a[:], in_=sb0_d[0:128, :])
    nc.sync.dma_start(out=sc0b[:], in_=sb0_d[128:192, :])
    for i, (lo, hi) in enumerate([(0, 128), (128, 256), (256, 384),
                                  (384, 512), (512, 576)]):
        nc.sync.dma_start(out=sc1[i][:], in_=sb1_d[lo:hi, :])
    nc.sync.dma_start(out=b2a[:], in_=b2_d[0:128, :])
    nc.sync.dma_start(out=b2b[:], in_=b2_d[128:192, :])
    nc.sync.dma_start(
        out=tmps[:],
        in_=bass.AP(tensor=tmp_d, offset=0, ap=[[0, 128], [1, 1]]))

    ident = P.tile([128, 128], BF16, tag="ident")
    make_identity(nc, ident[:])
    identf = P.tile([128, 128], F32, tag="identf")
    make_identity(nc, identf[:])

    # staging/state tiles
    xpool = P.tile([128, YR * WP], BF16, tag="xpool")   # later reused as oa
    plh = P.tile([128, YR, 48], BF16, tag="plh")
    pl = P.tile([128, 26, 48], BF16, tag="pl")
    vint = P.tile([128, TR, 48], BF16, tag="vint")
    tscr = P.tile([128, TR, 48], BF16, tag="tscr")
    brs = P.tile([128, 51 * WP], BF16, tag="brs")       # later reused as ob
    Ts = [P.tile([T_SPECS[i], TR * WP], BF16, tag=f"T{i}", name=f"T{i}")
          for i in range(5)]
    qk = [P.tile([128, SR * W], BF16, tag=f"qk{i}", name=f"qk{i}")
          for i in range(3)]
    v0 = P.tile([128, QR * W], BF16, tag="v0")
    v1 = P.tile([64, QR * W], BF16, tag="v1")
    sqac = [P.tile([128, 10], F32, tag=f"sqac{i}", name=f"sqac{i}")
            for i in range(3)]
    accs = [P.tile([128, 1], F32, tag=f"accs{i}", name=f"accs{i}")
            for i in range(3)]
    ssa = P.tile([128, 194], F32, tag="ssa")
    ssb = P.tile([64, 194], F32, tag="ssb")
    sfa = P.tile([128, 194], F32, tag="sfa")
    sfb = P.tile([64, 194], F32, tag="sfb")
    rska = P.tile([128, 1], F32, tag="rska")
    rskb = P.tile([64, 1], F32, tag="rskb")
    rqa = P.tile([128, 1], F32, tag="rqa")
    rqb = P.tile([64, 1], F32, tag="rqb")
    sta = P.tile([128, C], F32, tag="sta")
    stb = P.tile([64, C], F32, tag="stb")
    paf = P.tile([128, C], BF16, tag="paf")
    pbf = P.tile([64, C], BF16, tag="pbf")
    pta = P.tile([128, C], BF16, tag="pta")
    ptb = P.tile([64, C], BF16, tag="ptb")

    xfv = xfold.rearrange("p (r w) -> p r w", w=WP)
    xpv = xpool.rearrange("p (r w) -> p r w", w=WP)
    brv = brs.rearrange("p (r w) -> p r w", w=WP)
    Tv = [t.rearrange("p (r w) -> p r w", w=WP) for t in Ts]
    qkv = [t.rearrange("p (r w) -> p r w", w=W) for t in qk]
    v0v = v0.rearrange("p (r w) -> p r w", w=W)
    v1v = v1.rearrange("p (r w) -> p r w", w=W)
    w0fv = w0f.rearrange("p (t m) -> p t m", t=3)
    w0gv = w0g.rearrange("p (t m) -> p t m", t=3)
    w1v = [w.rearrange("p (t m) -> p t m", t=3) for w in w1s]
    w2av = w2da.rearrange("p (t m) -> p t m", t=9)
    w2fv = w2fb.rearrange("p (t m) -> p t m", t=3)
    w2gv = w2gb.rearrange("p (t m) -> p t m", t=3)

    # border memsets: col 0 of shifted x copy; T tiles row0 + cols 0/97;
    # brs row 0 + cols 0/97 (brs interior rows fully rewritten)
    nc.vector.memset(xfv[0:64, :, 0:1], 0.0)
    for i in range(5):
        p = T_SPECS[i]
        nc.vector.memset(Tv[i][0:p, 0:1, :], 0.0)
        nc.vector.memset(Tv[i][0:p, :, 0:1], 0.0)
        nc.vector.memset(Tv[i][0:p, :, 97:98], 0.0)
    nc.vector.memset(brv[:, 0:1, :], 0.0)
    nc.vector.memset(brv[:, :, 0:1], 0.0)
    nc.vector.memset(brv[:, :, 97:98], 0.0)

    # ---------------- conv0 ----------------
    # out-tile A (128): x1|x3 -> xpool; out-tile B (64): x2 -> T slots
    blocksA = _row_blocks(YR)
    blocksB = _row_blocks(TR)
    with tc.tile_pool(name="ps_c0", bufs=4, space="PSUM") as pp0:
        for (r0, nr) in blocksA:
            ps = pp0.tile([128, 5, W], F32, tag="c0psA")
            for dy in range(3):
                nc.tensor.matmul(
                    ps[:, 0:nr, :], lhsT=w0fv[:, dy, 0:128],
                    rhs=xfv[:, r0 + dy:r0 + dy + nr, 1:97],
                    start=(dy == 0), stop=False)
                nc.tensor.matmul(
                    ps[:, 0:nr, :], lhsT=w0gv[64:128, dy, 0:128],
                    rhs=xfv[64:128, r0 + dy:r0 + dy + nr, 2:98],
                    start=False, stop=(dy == 2))
            nc.scalar.activation(
                out=xpv[:, r0:r0 + nr, 1:97], in_=ps[:, 0:nr, :],
                func=mybir.ActivationFunctionType.Relu,
                bias=sc0a[:, 1:2], scale=sc0a[:, 0:1])
        for (r0, nr) in blocksB:
            ps = pp0.tile([128, 5, W], F32, tag="c0psB")
            for dy in range(3):
                nc.tensor.matmul(
                    ps[0:64, 0:nr, :], lhsT=w0fv[:, dy, 128:192],
                    rhs=xfv[:, r0 + dy:r0 + dy + nr, 1:97],
                    start=(dy == 0), stop=False)
                nc.tensor.matmul(
                    ps[0:64, 0:nr, :], lhsT=w0gv[64:128, dy, 128:192],
                    rhs=xfv[64:128, r0 + dy:r0 + dy + nr, 2:98],
                    start=False, stop=(dy == 2))
            # x2 = t channels 64..127 -> ky1 slot T2[0:64]; the ky0/ky2 slots
            # (T0[64:]/T3[64:]) are bulk row-shifted DMA copies of it below
            nc.scalar.activation(
                out=Tv[2][0:64, r0:r0 + nr, 1:97], in_=ps[0:64, 0:nr, :],
                func=mybir.ActivationFunctionType.Relu,
                bias=sc0b[:, 1:2], scale=sc0b[:, 0:1])

    # x2 ky-shifted slot copies (chunked; T2 col pads are zero so full width)
    for (a, b) in ((0, 30), (30, 49)):
        nc.sync.dma_start(out=Tv[0][64:128, a + 1:b + 1, :],
                          in_=Tv[2][0:64, a:b, :])
        nc.sync.dma_start(out=Tv[3][64:128, a:b, :],
                          in_=Tv[2][0:64, a + 1:b + 1, :])

    # ---------------- pools + bilinear (chunked over pl rows) ----------------
    cA = P.tile([128, 2], F32, tag="cA")
    nc.vector.memset(cA[0:64, 0:1], 0.75)
    nc.vector.memset(cA[0:64, 1:2], 0.25)
    nc.vector.memset(cA[64:128, 0:1], 0.1875)
    nc.vector.memset(cA[64:128, 1:2], 0.0625)
    cC = P.tile([128, 1], F32, tag="cC")
    nc.vector.memset(cC[0:64, :], 1.0)
    nc.vector.memset(cC[64:128, :], 0.25)

    pl_chunks = [(0, 5), (5, 10), (10, 15), (15, 20), (20, 25), (25, 26)]
    kv_o = 0   # next odd-row k (vint[2k+1], k<=24)
    kv_e = 0   # next even-row k (vint[2k+2], k<=23)
    hv = 0     # next t-row for the horizontal pass

    def hpass(a, b):
        if b <= a:
            return
        nc.vector.tensor_scalar(out=brv[:, 1 + a:1 + b, 1:2],
                                in0=vint[:, a:b, 0:1], scalar1=cC[:, 0:1],
                                scalar2=None, op0=mybir.AluOpType.mult)
        nc.vector.tensor_scalar(out=brv[:, 1 + a:1 + b, 96:97],
                                in0=vint[:, a:b, 47:48], scalar1=cC[:, 0:1],
                                scalar2=None, op0=mybir.AluOpType.mult)
        nc.vector.tensor_scalar(out=tscr[:, a:b, 0:47], in0=vint[:, a:b, 1:48],
                                scalar1=cA[:, 1:2], scalar2=None,
                                op0=mybir.AluOpType.mult)
        nc.vector.scalar_tensor_tensor(
            out=brv[:, 1 + a:1 + b, 2:96:2], in0=vint[:, a:b, 0:47],
            scalar=cA[:, 0:1], in1=tscr[:, a:b, 0:47],
            op0=mybir.AluOpType.mult, op1=mybir.AluOpType.add)
        nc.vector.tensor_scalar(out=tscr[:, a:b, 0:47], in0=vint[:, a:b, 1:48],
                                scalar1=cA[:, 0:1], scalar2=None,
                                op0=mybir.AluOpType.mult)
        nc.vector.scalar_tensor_tensor(
            out=brv[:, 1 + a:1 + b, 3:96:2], in0=vint[:, a:b, 0:47],
            scalar=cA[:, 1:2], in1=tscr[:, a:b, 0:47],
            op0=mybir.AluOpType.mult, op1=mybir.AluOpType.add)

    for (k0, k1) in pl_chunks:
        # horizontal pool pairs for y rows 2k0..2k1-1
        nc.vector.tensor_tensor(out=plh[0:64, 2 * k0:2 * k1, :],
                                in0=xpv[0:64, 2 * k0:2 * k1, 1:97:2],
                                in1=xpv[0:64, 2 * k0:2 * k1, 2:98:2],
                                op=mybir.AluOpType.max)
        nc.vector.tensor_tensor(out=plh[64:128, 2 * k0:2 * k1, :],
                                in0=xpv[64:128, 2 * k0:2 * k1, 1:97:2],
                                in1=xpv[64:128, 2 * k0:2 * k1, 2:98:2],
                                op=mybir.AluOpType.add)
        # vertical pool pairs -> pl rows k0..k1-1
        nc.vector.tensor_tensor(out=pl[0:64, k0:k1, :],
                                in0=plh[0:64, 2 * k0:2 * k1:2, :],
                                in1=plh[0:64, 2 * k0 + 1:2 * k1:2, :],
                                op=mybir.AluOpType.max)
        nc.vector.tensor_tensor(out=pl[64:128, k0:k1, :],
                                in0=plh[64:128, 2 * k0:2 * k1:2, :],
                                in1=plh[64:128, 2 * k0 + 1:2 * k1:2, :],
                                op=mybir.AluOpType.add)
        if k0 == 0:
            nc.vector.tensor_copy(out=vint[:, 0, :], in_=pl[:, 0, :])
        # vertical bilinear rows that only need pl rows < k1
        ke_o = min(k1 - 1, 25)
        if ke_o > kv_o:
            a, b = kv_o, ke_o
            nc.vector.tensor_scalar(out=tscr[:, a:b, :], in0=pl[:, a + 1:b + 1, :],
                                    scalar1=0.25, scalar2=None,
                                    op0=mybir.AluOpType.mult)
            nc.vector.scalar_tensor_tensor(
                out=vint[:, 2 * a + 1:2 * b:2, :], in0=pl[:, a:b, :],
                scalar=0.75, in1=tscr[:, a:b, :],
                op0=mybir.AluOpType.mult, op1=mybir.AluOpType.add)
            kv_o = ke_o
        ke_e = min(k1 - 1, 24)
        if ke_e > kv_e:
            a, b = kv_e, ke_e
            nc.vector.tensor_scalar(out=tscr[:, a:b, :], in0=pl[:, a + 1:b + 1, :],
                                    scalar1=0.75, scalar2=None,
                                    op0=mybir.AluOpType.mult)
            nc.vector.scalar_tensor_tensor(
                out=vint[:, 2 * a + 2:2 * b + 1:2, :], in0=pl[:, a:b, :],
                scalar=0.25, in1=tscr[:, a:b, :],
                op0=mybir.AluOpType.mult, op1=mybir.AluOpType.add)
            kv_e = ke_e
        # horizontal pass over fully-available vint rows
        avail = min(2 * kv_o + 1, 2 * kv_e + 2) if k1 < 26 else TR
        hpass(hv, avail)
        hv = avail

    # brs rows 0..50 (row 0 zero, rows 1..50 = t rows 0..49) -> T slots
    # (ky slot s stores t row rho at tile row rho+1-s). Row-chunked so
    # conv1's early blocks start on the first half.
    for (a, b) in ((0, 10), (10, 30), (30, 50)):
        nc.sync.dma_start(out=Tv[0][0:64, a:b, :], in_=brv[0:64, a:b, :])
        nc.sync.dma_start(out=Tv[1][0:64, a:b, :], in_=brv[64:128, a:b, :])
        nc.sync.dma_start(out=Tv[1][64:128, a:b, :],
                          in_=brv[0:64, a + 1:b + 1, :])
        nc.sync.dma_start(out=Tv[2][64:128, a:b, :],
                          in_=brv[64:128, a + 1:b + 1, :])
        bb = min(b, 49)
        nc.sync.dma_start(out=Tv[3][0:64, a:bb, :],
                          in_=brv[0:64, a + 2:bb + 2, :])
        nc.sync.dma_start(out=Tv[4][0:64, a:bb, :],
                          in_=brv[64:128, a + 2:bb + 2, :])

    # oa (= xpool reuse) and ob (= brs reuse, kx-folded: p0..63 = ob shifted
    # right one col, p64..127 = ob) borders for the depthwise reads
    oav = xpv[:, 0:50, :]
    nc.vector.memset(oav[:, 0:1, :], 0.0)
    nc.vector.memset(oav[:, :, 0:1], 0.0)
    nc.vector.memset(oav[:, :, 97:98], 0.0)
    obv = brv[:, 0:50, :]
    nc.vector.memset(obv[0:64, :, 1:2], 0.0)

    # ---------------- conv1 + attention prologue ----------------
    qk_blocks = _row_blocks(SR)
    v_blocks = _row_blocks(QR)

    def conv1_tile(ot, blocks, dst, dstv, sq):
        mw = T_SPECS[ot] if ot == 4 else 128
        m0 = 128 * ot
        for bi, (r0, nr) in enumerate(blocks):
            ps = pp1.tile([128, 5, W], F32, tag="c1ps")
            first = True
            for ti in range(5):
                tp = T_SPECS[ti]
                for dx in range(3):
                    nc.tensor.matmul(
                        ps[0:mw, 0:nr, :],
                        lhsT=w1v[ti][:, dx, m0:m0 + mw],
                        rhs=Tv[ti][0:tp, r0:r0 + nr, dx:dx + 96],
                        start=first, stop=(ti == 4 and dx == 2))
                    first = False
            nc.scalar.activation(
                out=dstv[0:mw, r0:r0 + nr, :], in_=ps[0:mw, 0:nr, :],
                func=mybir.ActivationFunctionType.Relu,
                bias=sc1[ot][:, 1:2], scale=sc1[ot][:, 0:1])
            if sq is not None:
                dump = ev.tile([128, 5 * W], BF16, tag="sqd")
                nc.scalar.activation(
                    out=dump[:, 0:nr * W],
                    in_=dst[:, r0 * W:(r0 + nr) * W],
                    func=mybir.ActivationFunctionType.Square,
                    accum_out=sq[:, bi:bi + 1])

    with tc.tile_pool(name="ps_c1", bufs=3, space="PSUM") as pp1, \
         tc.tile_pool(name="ps_tr", bufs=2, space="PSUM") as ppt, \
         tc.tile_pool(name="ps_s", bufs=1, space="PSUM") as pps:

        for ot in range(3):
            conv1_tile(ot, qk_blocks, qk[ot], qkv[ot], sqac[ot])

        # transposes + unnormalized S'^T partial (rows = k channels)
        qk0r = qk[0].rearrange("p (c k) -> p c k", k=128)
        qk1r = qk[1].rearrange("p (c k) -> p c k", k=128)
        qk2r = qk[2].rearrange("p (c k) -> p c k", k=128)
        sp = pps.tile([128, 2 * C], F32, tag="sp")
        for g in range(NCH // 3):
            tq = ppt.tile([128, 3 * C], BF16, tag="tq")
            tk = ppt.tile([128, 3 * C], BF16, tag="tk")
            tqv = tq.rearrange("p (i c) -> p i c", c=C)
            tkv = tk.rearrange("p (i c) -> p i c", c=C)
            for i in range(3):
                ci = 3 * g + i
                nc.tensor.transpose(tqv[:, i, 0:128], qk0r[:, ci, :], ident[:])
                nc.tensor.transpose(tqv[:, i, 128:192], qk1r[0:64, ci, :],
                                    ident[0:64, 0:64])
                nc.tensor.transpose(tkv[:, i, 0:64], qk1r[64:128, ci, :],
                                    ident[64:128, 64:128])
                nc.tensor.transpose(tkv[:, i, 64:192], qk2r[:, ci, :], ident[:])
            qtc = ev.tile([128, 3 * C], BF16, tag="qtc")
            ktc = ev.tile([128, 3 * C], BF16, tag="ktc")
            nc.scalar.copy(out=qtc[:], in_=tq[:])
            nc.scalar.copy(out=ktc[:], in_=tk[:])
            qcv = qtc.rearrange("p (i c) -> p i c", c=C)
            kcv = ktc.rearrange("p (i c) -> p i c", c=C)
            for i in range(3):
                nc.tensor.matmul(sp[:, 0:C], lhsT=kcv[:, i, 0:128],
                                 rhs=qcv[:, i, :],
                                 start=(g == 0 and i == 0),
                                 stop=(g == NCH // 3 - 1 and i == 2))
                nc.tensor.matmul(sp[0:64, C:2 * C], lhsT=kcv[:, i, 128:192],
                                 rhs=qcv[:, i, :],
                                 start=(g == 0 and i == 0),
                                 stop=(g == NCH // 3 - 1 and i == 2))

        # sumsq totals + AllReduce staging
        for ti in range(3):
            nc.vector.reduce_sum(out=accs[ti][:], in_=sqac[ti][:],
                                 axis=mybir.AxisListType.X)
        nc.vector.tensor_copy(out=ssa[:, 192:193], in_=accs[0][:])
        nc.vector.tensor_copy(out=ssb[:, 192:193], in_=accs[1][0:64, :])
        nc.sync.dma_start(out=ssa[0:64, 193:194], in_=accs[1][64:128, :])
        nc.sync.dma_start(out=ssa[64:128, 193:194], in_=accs[2][0:64, :])
        nc.sync.dma_start(out=ssb[0:64, 193:194], in_=accs[2][64:128, :])
        nc.scalar.copy(out=ssa[:, 0:192], in_=sp[:, 0:C])
        nc.scalar.copy(out=ssb[:, 0:192], in_=sp[0:64, C:2 * C])
        nc.gpsimd.dma_start(out=cc2i[0:128, :], in_=ssa[:])
        nc.gpsimd.dma_start(out=cc2i[128:192, :], in_=ssb[:])
        nc.gpsimd.collective_compute(
            "AllReduce", mybir.AluOpType.add, replica_groups=GROUPS,
            ins=[cc2i[:]], outs=[cc2o[:]])
        nc.gpsimd.dma_start(out=sfa[:], in_=cc2o[0:128, :])
        nc.gpsimd.dma_start(out=sfb[:], in_=cc2o[128:192, :])

        # conv1 v tiles overlap the collective
        conv1_tile(3, v_blocks, v0, v0v, None)
        conv1_tile(4, v_blocks, v1, v1v, None)

    # ---------------- normalize logits + softmax + P^T ----------------
    halves = ((sfa, rska, rqa, sta, 128), (sfb, rskb, rqb, stb, 64))
    for (sf, rsk, rq, st, p) in halves:
        nc.scalar.activation(out=rsk[:], in_=sf[0:p, 193:194],
                             func=mybir.ActivationFunctionType.Sqrt)
        nc.scalar.activation(out=rq[:], in_=sf[0:p, 192:193],
                             func=mybir.ActivationFunctionType.Sqrt)
    for (sf, rsk, rq, st, p) in halves:
        for r in (rsk, rq):
            nc.vector.tensor_scalar(out=r[:], in0=r[:], scalar1=1e-12,
                                    scalar2=None, op0=mybir.AluOpType.max)
            nc.vector.reciprocal(out=r[:], in_=r[:])
    for (sf, rsk, rq, st, p) in halves:
        # scale S'^T rows (k channels) by 1/|k|
        nc.vector.tensor_scalar(out=st[:], in0=sf[0:p, 0:192], scalar1=rsk[:],
                                scalar2=None, op0=mybir.AluOpType.mult)
        nc.vector.tensor_tensor(out=rq[:], in0=rq[:], in1=tmps[0:p, :],
                                op=mybir.AluOpType.mult)

    with tc.tile_pool(name="ps_pt", bufs=1, space="PSUM") as ppm, \
         tc.tile_pool(name="ps_pv", bufs=2, space="PSUM") as ppv:
        sps1 = ppm.tile([128, C], F32, tag="sps1")
        nc.tensor.transpose(sps1[:, 0:128], sta[:, 0:128], identf[:])
        nc.tensor.transpose(sps1[:, 128:192], stb[:, 0:128],
                            identf[0:64, 0:64])
        sps2 = ppm.tile([64, C], F32, tag="sps2")
        nc.tensor.transpose(sps2[0:64, 0:128], sta[:, 128:192], identf[:])
        nc.tensor.transpose(sps2[0:64, 128:192], stb[:, 128:192],
                            identf[0:64, 0:64])

        for (sps, rq, pf, p) in ((sps1, rqa, paf, 128), (sps2, rqb, pbf, 64)):
            mx = ev.tile([128, 1], F32, tag="mx")
            nb = ev.tile([128, 1], F32, tag="nb")
            sm = ev.tile([128, 1], F32, tag="sm")
            pexp = ev.tile([128, C], F32, tag="pexp")
            nc.vector.reduce_max(out=mx[0:p, :], in_=sps[0:p, :],
                                 axis=mybir.AxisListType.X)
            nc.vector.scalar_tensor_tensor(
                out=nb[0:p, :], in0=mx[0:p, :], scalar=-1.0, in1=rq[:],
                op0=mybir.AluOpType.mult, op1=mybir.AluOpType.mult)
            nc.scalar.activation(out=pexp[0:p, :], in_=sps[0:p, :],
                                 func=mybir.ActivationFunctionType.Exp,
                                 bias=nb[0:p, :], scale=rq[:],
                                 accum_out=sm[0:p, :])
            nc.vector.reciprocal(out=sm[0:p, :], in_=sm[0:p, :])
            nc.vector.tensor_scalar(out=pf[:], in0=pexp[0:p, :],
                                    scalar1=sm[0:p, :], scalar2=None,
                                    op0=mybir.AluOpType.mult)

        tp1 = ppm.tile([128, C], BF16, tag="tp1")
        nc.tensor.transpose(tp1[:, 0:128], paf[:, 0:128], ident[:])
        nc.tensor.transpose(tp1[:, 128:192], pbf[:, 0:128], ident[0:64, 0:64])
        nc.scalar.copy(out=pta[:], in_=tp1[:])
        tp2 = ppm.tile([64, C], BF16, tag="tp2")
        nc.tensor.transpose(tp2[0:64, 0:128], paf[:, 128:192], ident[:])
        nc.tensor.transpose(tp2[0:64, 128:192], pbf[:, 128:192],
                            ident[0:64, 0:64])
        nc.scalar.copy(out=ptb[:], in_=tp2[0:64, :])

        # out = P @ v
        for (r0, nr) in v_blocks:
            po = ppv.tile([128, 5, W], F32, tag="po")
            po2 = ppv.tile([128, 5, W], F32, tag="po2")
            nc.tensor.matmul(po[:, 0:nr, :], lhsT=pta[:, 0:128],
                             rhs=v0v[:, r0:r0 + nr, :], start=True, stop=False)
            nc.tensor.matmul(po[:, 0:nr, :], lhsT=ptb[:, 0:128],
                             rhs=v1v[:, r0:r0 + nr, :], start=False, stop=True)
            nc.tensor.matmul(po2[0:64, 0:nr, :], lhsT=pta[:, 128:192],
                             rhs=v0v[:, r0:r0 + nr, :], start=True, stop=False)
            nc.tensor.matmul(po2[0:64, 0:nr, :], lhsT=ptb[:, 128:192],
                             rhs=v1v[:, r0:r0 + nr, :], start=False, stop=True)
            nc.scalar.copy(out=oav[:, r0 + 1:r0 + 1 + nr, 1:97],
                           in_=po[:, 0:nr, :])
            nc.scalar.copy(out=obv[0:64, r0 + 1:r0 + 1 + nr, 2:98],
                           in_=po2[0:64, 0:nr, :])
        # replicate ob to partitions 64..127 at the unshifted column offset
        nc.sync.dma_start(out=obv[64:128, 1:50, 1:97],
                          in_=obv[0:64, 1:50, 2:98])

    # ---------------- depthwise conv + bias ----------------
    yv = yout.rearrange("c (r w) -> c r w", w=W)
    with tc.tile_pool(name="ps_dw", bufs=4, space="PSUM") as ppd:
        for (r0, nr) in _row_blocks(OR_):
            ps = ppd.tile([128, 5, W], F32, tag="dwps")
            for t in range(9):
                dy, dx = t // 3 - 1, t % 3 - 1
                nc.tensor.matmul(
                    ps[:, 0:nr, :],
                    lhsT=w2av[:, t, :],
                    rhs=oav[:, r0 + 1 + dy:r0 + 1 + dy + nr, 1 + dx:97 + dx],
                    start=(t == 0), stop=(t == 8))
            fo = ev.tile([128, 5, W], F32, tag="fo")
            nc.scalar.activation(out=fo[:, 0:nr, :], in_=ps[:, 0:nr, :],
                                 func=mybir.ActivationFunctionType.Identity,
                                 bias=b2a[:, 0:1], scale=1.0)
            nc.sync.dma_start(out=yv[0:128, r0:r0 + nr, :],
                              in_=fo[:, 0:nr, :])
        for (r0, nr) in _row_blocks(OR_):
            ps = ppd.tile([128, 5, W], F32, tag="dwps")
            for dy in range(3):
                nc.tensor.matmul(
                    ps[0:64, 0:nr, :], lhsT=w2fv[:, dy, :],
                    rhs=obv[:, r0 + dy:r0 + dy + nr, 1:97],
                    start=(dy == 0), stop=False)
                nc.tensor.matmul(
                    ps[0:64, 0:nr, :], lhsT=w2gv[64:128, dy, :],
                    rhs=obv[64:128, r0 + dy:r0 + dy + nr, 2:98],
                    start=False, stop=(dy == 2))
            fo = ev.tile([128, 5, W], F32, tag="fo")
            nc.scalar.activation(out=fo[0:64, 0:nr, :], in_=ps[0:64, 0:nr, :],
                                 func=mybir.ActivationFunctionType.Identity,
                                 bias=b2b[:, 0:1], scale=1.0)
            nc.sync.dma_start(out=yv[128:192, r0:r0 + nr, :],
                              in_=fo[0:64, 0:nr, :])
    ctx.close()


# ---------------- host side ----------------
_NC_CACHE = None


def _get_nc():
    global _NC_CACHE
    if _NC_CACHE is None:
        _NC_CACHE = build_nc()
    return _NC_CACHE


def _pack_weights(inp, flip):
    bf = ml_dtypes.bfloat16
    w0 = inp["w0"][:, :, ::-1, :] if flip else inp["w0"]
    w1 = inp["w1"][:, :, ::-1, :] if flip else inp["w1"]
    w2 = inp["w2"][:, :, ::-1, :] if flip else inp["w2"]
    w0 = np.asarray(w0, np.float32)
    w1 = np.asarray(w1, np.float32)
    w2 = np.asarray(w2, np.float32)

    # conv0: out-channel order [x1(0:64), x3(128:192), x2(64:128)]
    cho = np.concatenate([np.arange(0, 64), np.arange(128, 192),
                          np.arange(64, 128)])
    w0p = w0[cho]                       # [192, 64, 3, 3]
    w0f = np.zeros((128, 3, C), np.float32)
    w0f[0:64] = w0p[:, :, :, 0].transpose(1, 2, 0)    # (c, kx=0)
    w0f[64:128] = w0p[:, :, :, 1].transpose(1, 2, 0)  # (c, kx=1)
    w0g = np.zeros((128, 3, C), np.float32)           # data at partitions 64..
    w0g[64:128] = w0p[:, :, :, 2].transpose(1, 2, 0)
    s0 = inp["g0"] / np.sqrt(inp["v0"] + BN_EPS)
    t0 = inp["be0"] + (inp["b0"] - inp["m0"]) * s0
    sb0 = np.stack([s0, t0], axis=1).astype(np.float32)[cho]

    # conv1: contraction partition P = ky*192 + c, tiles of 128
    w1tiles = []
    bounds = [0, 128, 256, 384, 512, 576]
    for i in range(5):
        Pr = np.arange(bounds[i], bounds[i + 1])
        cc, ky = Pr % C, Pr // C
        wt = w1[:, cc, ky, :].transpose(1, 2, 0)   # [np, 3, 576]
        w1tiles.append(np.ascontiguousarray(wt.reshape(len(Pr), 3 * C3)))
    s1 = inp["g1"] / np.sqrt(inp["v1"] + BN_EPS)
    t1 = inp["be1"] + (inp["b1"] - inp["m1"]) * s1
    sb1 = np.stack([s1, t1], axis=1).astype(np.float32)

    w2da = np.zeros((128, 9, 128), np.float32)
    w2fb = np.zeros((128, 3, 64), np.float32)
    w2gb = np.zeros((128, 3, 64), np.float32)
    r64, r128 = np.arange(64), np.arange(128)
    for t in range(9):
        d = w2[:, 0, t // 3, t % 3]
        w2da[r128, t, r128] = d[0:128]
    for dy in range(3):
        db = w2[128:192, 0, dy, :]
        w2fb[r64, dy, r64] = db[:, 0]
        w2fb[64 + r64, dy, r64] = db[:, 1]
        w2gb[64 + r64, dy, r64] = db[:, 2]

    out = {
        "w0f": np.ascontiguousarray(w0f.reshape(128, 3 * C)).astype(bf),
        "w0g": np.ascontiguousarray(w0g.reshape(128, 3 * C)).astype(bf),
        "sb0p": sb0,
        "sb1": sb1,
        "w2da": np.ascontiguousarray(w2da.reshape(128, 9 * 128)).astype(bf),
        "w2fb": np.ascontiguousarray(w2fb.reshape(128, 3 * 64)).astype(bf),
        "w2gb": np.ascontiguousarray(w2gb.reshape(128, 3 * 64)).astype(bf),
        "b2v": np.asarray(inp["b2"], np.float32).reshape(C, 1),
    }
    for i in range(5):
        out[f"w1t{i}"] = w1tiles[i].astype(bf)
    return out


def kernel(**inputs):
    inputs = {k: np.asarray(v) for k, v in inputs.items()}
    x = inputs["x"]
    B = x.shape[0]
    bf = ml_dtypes.bfloat16
    packs = [_pack_weights(inputs, flip) for flip in (False, True)]
    tempv = np.asarray(inputs["temp"], np.float32).reshape(1, 1)

    in_maps = []
    for core in range(8):
        s, h = core // 2, core % 2
        xi = np.asarray(x[s], np.float32)
        if h:
            xi = xi[:, ::-1, :]
        slab = np.zeros((64, XR, WP), np.float32)
        slab[:, 1:54, 1:97] = xi[:, 0:53, :]
        m = dict(packs[h])
        m["xs"] = np.ascontiguousarray(slab.reshape(64, XR * WP)).astype(bf)
        m["tempv"] = tempv
        in_maps.append(m)

    nc = _get_nc()
    res = run_bass_kernel_spmd(nc, in_maps, list(range(8)))
    out = np.zeros((B, C, 96, 96), np.float32)
    for core in range(8):
        s, h = core // 2, core % 2
        yc = res.results[core]["yout"].reshape(C, OR_, W)
        if h:
            out[s, :, 48:96] = yc[:, ::-1, :]
        else:
            out[s, :, 0:48] = yc
    return out
